# revision 27
# baseline (speedup 1.0000x reference)
"""Trainium2 Bass kernel for quantum-projection multi-head self-attention.

Reference computation (per batch b, head h, with D = 64, H = 16):
    proj = cos(x_heads + theta)                         # [S, D]
    G    = proj @ proj.T / sqrt(D)                      # [S, S]  (symmetric!)
    attn = softmax(G, axis=-1) @ proj                   # [S, D]

Sharding: the 64 (b, h) pairs are data-parallel; 8 pairs per NeuronCore.

Device-side plan per head (S = 2048, D = 64), ~331 us vs the 393 us
baseline on the same harness:
  1. Host folds theta' = theta + pi/2 into x and ships it twice:
     natural [S, D] (s mod 128 partitions) AND transposed with
     duplicated partitions xT_dup [128, S] (partition p = row d mod
     64). The T layout eliminates all PE transposes, their PSUM->SBUF
     copies and the partition-duplication DMA of the old design.
  2. Custom DVE ops (both proven <= 8 ALU stages, short bypass taps;
     longer-lived intermediates and a 2-input PREP variant fault the
     HW DVE even though CoreSim accepts them):
       PREP: y = x' - 2pi*round(x'/2pi) -> fp16  (range reduction)
       SINP: sin(y) ~ y(1 + s(c3 + s(c5 + s c7))), s = y^2 (fp16 in,
             2x DVE port mode) -> ptx bf16 [128,S] / pvx fp8e4 [128,nd]
       EXPP: p ~ e^{g/256}-1 deg-3 (reads PSUM fp32) -> fp16
       EXPS: E = ((1+p)^2 gamma)^16 = e^{g/8-4} -> fp8e4, row-sum Z
             via accum_out for free
  3. QK: G[si] = ptx.T @ ptx per 128-row query block, two K=64 row
     groups concurrently on the PE, into a 3-deep rotation of
     [128, 1024] PSUM half-slab pool tiles (6 banks; PV owns 2).
     Separate pool tiles per slot are essential: a single manually
     sliced PSUM tile is dependency-tracked as one object and
     degrades the pipeline to lock-step (+250 us).
  4. exp per half-slab, interleaved 1:2 between DVE (2-pass custom,
     holds the slot only for pass1) and ACT (HW Exp + accumulator
     read for Z). The 1/3 spread keeps consecutive slots on
     different engines; consecutive same-engine slots serialize the
     slot stream.
  5. PV: fp8e4 DoubleRow matmuls contract two 128-key tiles per
     instruction (rhs = E pairs [128, 2, S] written directly by the
     exps): attnT[64, 512] += pvx[:, 2j:2j+2, :].T @ Epair. 2x PE
     throughput, half the instructions. M = 64 only: dual-fp8
     ldweights restricts lhsT free size to <= 128, which is why Z
     rides accum_out instead of a ones column.
  6. DVE copies PSUM->SBUF; outputs (attnT + Z halves) leave via the
     idle GPSIMD queue so their sem waits never block the SP queue
     that feeds input loads. Host divides by Z and transposes.

Emission is software-pipelined one head deep (QK+exp of head h before
PV of head h-1, PV as a contiguous block - interleaving PV into the
QK stream head-of-line-blocks the in-order PE queue). Sins are
batched per GROUP heads with their input DMAs fused per head.
"""

import math
from contextlib import ExitStack

import numpy as np

import concourse.bass as bass
import concourse.mybir as mybir
import concourse.tile as tile
from concourse import bacc

AF = mybir.ActivationFunctionType
ALU = mybir.AluOpType

B, S, E = 4, 2048, 1024
H = 16
D = E // H          # 64
N_CORES = 8
HEADS_PER_CORE = (B * H) // N_CORES  # 8

P = 128             # partitions
MAGIC = 1.5 * 2.0**23   # fp32 round-to-nearest trick constant
TWO_PI = 2.0 * math.pi
SHIFT = 4.0         # exp(g/8 - SHIFT); host multiplies back via Z division

_OPS = None


def _register_ops():
    """Register custom DVE ops at runtime (idempotent).

    PREP_ANTK : u = w - round(w), w = Src0*C0 + Src1  (5 ALU stages)
    EXPP_ANTK : p = ((C0 g + C1) g + C2) g  ~ e^{g/256}-1  (5 stages)
    EXPS_ANTK : out = ((1+p)^2 * C1)^16, accum_out = row sum  (7 stages)
    """
    global _OPS
    if _OPS is not None:
        return _OPS
    import concourse.dve_ops as dops
    from concourse.dve_spec import Spec, Src0, Src1, C0, C1, C2, sq, lower
    from concourse.dve_spec import _has_src1 as has_src1
    from concourse.dve_uop import DveOpSpec, AluOp as UAluOp

    def make_op(name, spec):
        if name in dops._SUB_OPCODE_FOR_NAME:
            return next(o for o in dops.OPS if o.name == name)
        row = 1 + len(dops.OPS)
        assert row < 0x20
        dops._SUB_OPCODE_FOR_NAME[name] = row
        shas = {}
        for ver in ("v3", "v4"):
            uops = lower(spec, ver=ver)
            shas[ver] = DveOpSpec(
                name=name, opcode=row, uops=uops, rd1_en=has_src1(spec)
            ).sha(ver)
        op = dops.DveOp(name, spec, subdim=False, uops_sha=shas)
        dops.OPS.append(op)
        dops.CUSTOM_DVE_SPECS[name] = spec
        return op

    def ref_prep(in0, in1, s0, s1, imm2):
        w = in0.astype(np.float32) * s0
        r = (w + s1) - s1
        return in0.astype(np.float32) - imm2 * r

    # theta is folded into x host-side (x' = x + theta + pi/2), so this
    # is pure range reduction: y = x' - 2pi*round(x'/2pi) in [-pi, pi].
    prep = make_op(
        "PREP_ANTK3",
        Spec(body=Src0 - (((Src0 * C0) + C1) - C1) * C2, reference=ref_prep),
    )

    def ref_sinp(in0, in1, s0, s1, imm2):
        y = in0.astype(np.float32)
        s = y * y
        return y * ((((s0 * s) + s1) * s + imm2) * s + 1.0)

    # sin(y) ~ y*(1 + s*(c3 + s*(c5 + s*c7))), s = y^2, y in [-pi, pi]
    _s = sq(Src0)
    sinp = make_op(
        "SINP_ANTK",
        Spec(body=(_s * ((C0 * _s + C1) * _s + C2)) * Src0 + Src0,
             reference=ref_sinp),
    )

    def ref_expp(in0, in1, s0, s1, imm2):
        g = in0.astype(np.float32)
        return ((s0 * g + s1) * g + imm2) * g

    expp = make_op(
        "EXPP_ANTK",
        Spec(body=((C0 * Src0 + C1) * Src0 + C2) * Src0, reference=ref_expp),
    )

    def ref_exps(in0, in1, s0, s1, imm2):
        q = in0.astype(np.float32) + s0
        q = q * q * s1
        for _ in range(4):
            q = q * q
        return q, q.sum(axis=-1, keepdims=True)

    exps = make_op(
        "EXPS_ANTK",
        Spec(body=sq(sq(sq(sq(sq(Src0 + C0) * C1)))),
             accum=UAluOp.ADD, reference=ref_exps),
    )
    _OPS = (prep, sinp, expp, exps)
    return _OPS


def _sin_poly_coeffs():
    """c7, c5, c3 for sin(y) ~ y*(1 + s*(c3 + s*(c5 + s*c7))), s=y^2."""
    y = np.linspace(-np.pi, np.pi, 20001)
    s = y * y
    mask = np.abs(y) > 1e-6
    A = np.stack([s[mask] ** 3, s[mask] ** 2, s[mask]], 1)
    b = np.sin(y[mask]) / y[mask] - 1.0
    cf, *_ = np.linalg.lstsq(A, b, rcond=None)
    return [float(v) for v in cf]   # c7, c5, c3


def _exp_poly_coeffs():
    """Deg-3 lstsq fit of e^{g/256}-1 (relative-error weighted), |g|<=66."""
    g = np.linspace(-66.0, 66.0, 4001)
    t = np.expm1(g / 256.0)
    A = np.stack([g**3, g**2, g], 1)
    w = 1.0 / np.exp(g / 256.0)
    cf, *_ = np.linalg.lstsq(A * w[:, None], t * w, rcond=None)
    return [float(v) for v in cf]   # c3, c2, c1


def build_core_program(s=S, d=D, heads=HEADS_PER_CORE, group=4,
                       dve_num=1, dve_mod=3, copy_on_act=False):
    """Build the single-core Bass program (same NEFF runs SPMD on all cores).

    Input DRAM tensors (theta + pi/2 is folded into x host-side):
      xs  : [heads, s, d] fp32    (natural per-head x + theta')
      xst : [heads, P, s] fp32    (same, transposed, d duplicated into 64..127)
    Outputs:
      out : [heads, d, s] fp32    (unnormalized attn^T)
      zs  : [heads, P, 2*(s//P)] fp32  (Z half-sums; host adds pairs)
    """
    n_sblk = s // P                   # 16 query blocks of 128 rows
    nd = n_sblk * d                   # 1024
    assert s % P == 0 and d == 64

    nc = bacc.Bacc("TRN2", target_bir_lowering=False, debug=False)

    xs = nc.dram_tensor("xs", [heads, s, d], mybir.dt.float32, kind="ExternalInput")
    xst = nc.dram_tensor("xst", [heads, P, s], mybir.dt.float32, kind="ExternalInput")
    out = nc.dram_tensor("out", [heads, d, s], mybir.dt.float32, kind="ExternalOutput")
    zs = nc.dram_tensor("zs", [heads, P, 2 * n_sblk], mybir.dt.float32,
                        kind="ExternalOutput")

    prep_op, sinp_op, expp_op, exps_op = _register_ops()
    C3p, C2p, C1p = _exp_poly_coeffs()
    S7, S5, S3 = _sin_poly_coeffs()
    GAMMA = math.exp(-SHIFT / 16.0)   # ((1+p)^2*GAMMA)^16 = e^{g/8-SHIFT}

    with tile.TileContext(nc) as tc, ExitStack() as ctx:
        const = ctx.enter_context(tc.tile_pool(name="const", bufs=1))
        sb = ctx.enter_context(tc.tile_pool(name="sb", bufs=2))
        epool = ctx.enter_context(tc.tile_pool(name="epool", bufs=2))
        ps = ctx.enter_context(tc.tile_pool(name="ps", bufs=1, space="PSUM"))

        shift_sb = const.tile([P, 1], mybir.dt.float32, tag="shift")
        nc.vector.memset(shift_sb, -SHIFT)

        # PSUM: 3 rotating QK half-slabs (6 banks) + 2 PV slots (2 banks);
        # separate pool tiles per slot so WAR tracking stays per-slot
        psPV = ps.tile([d, 2, 512], mybir.dt.float32, tag="pv", bufs=1)

        state = {}

        def emit_sin(h):
            xt_t = sb.tile([P, s], mybir.dt.float32, tag="xt", bufs=3)
            xv = xt_t.rearrange("p (q c) -> p q c", q=4)
            xr = xst[h].rearrange("p (q c) -> p q c", q=4)
            for q in range(4):
                nc.sync.dma_start(xv[:, q, :], xr[:, q, :])
            xn_t = sb.tile([P, nd], mybir.dt.float32, tag="xn", bufs=3)
            nv = xn_t.rearrange("p (n d) -> p n d", d=d)
            nr = xs[h].rearrange("(n p) d -> p n d", p=P)
            for q in range(4):
                nc.sync.dma_start(nv[:, q * 4:(q + 1) * 4, :],
                                  nr[:, q * 4:(q + 1) * 4, :])
            ut_t = sb.tile([P, s], mybir.dt.float16, tag="ut", bufs=2)
            nc.vector._custom_dve(
                prep_op, out=ut_t, in0=xt_t,
                s0=1.0 / TWO_PI, s1=MAGIC, imm2=TWO_PI)
            ptx = sb.tile([P, s], mybir.dt.bfloat16, tag="ptx", bufs=2)
            nc.vector._custom_dve(
                sinp_op, out=ptx, in0=ut_t, s0=S7, s1=S5, imm2=S3)
            un = sb.tile([P, nd], mybir.dt.float16, tag="un", bufs=2)
            nc.vector._custom_dve(
                prep_op, out=un, in0=xn_t,
                s0=1.0 / TWO_PI, s1=MAGIC, imm2=TWO_PI)
            pvx = sb.tile([P, nd], mybir.dt.float8e4, tag="pvx", bufs=2)
            nc.vector._custom_dve(
                sinp_op, out=pvx, in0=un, s0=S7, s1=S5, imm2=S3)
            state[h] = [ptx, pvx, None, None, None]

        ctr = [0]

        def emit_pv_chunk(h, idx):
            ptx, pvx, epairs, zsl, at = state[h]
            pvv = pvx.rearrange("p (t d) -> p t d", d=d)
            sb_i, tjp = idx // (n_sblk // 2), idx % (n_sblk // 2)
            slot = sb_i % 2
            nc.tensor.matmul(
                psPV[:, slot, :],
                pvv[:, 2 * tjp:2 * tjp + 2, :],
                epairs[tjp][:, :, sb_i * 512:(sb_i + 1) * 512],
                start=(tjp == 0), stop=(tjp == n_sblk // 2 - 1),
                perf_mode=mybir.MatmulPerfMode.DoubleRow)
            if tjp == n_sblk // 2 - 1:
                cp = nc.scalar.copy if copy_on_act else nc.vector.tensor_copy
                cp(at[:, sb_i * 512:(sb_i + 1) * 512], psPV[:, slot, :])
                if sb_i % 2 == 1:
                    c = sb_i // 2
                    nc.gpsimd.dma_start(
                        out[h, :, c * (s // 2):(c + 1) * (s // 2)],
                        at[:, c * (s // 2):(c + 1) * (s // 2)])
                if sb_i == 3:
                    nc.gpsimd.dma_start(zs[h], zsl)
                    del state[h]

        def emit_qk_exp(h):
            ptx, pvx, _, _, _ = state[h]
            epairs = []
            zsl = sb.tile([P, 2 * n_sblk], mybir.dt.float32, tag="z", bufs=2)
            at = sb.tile([d, s], mybir.dt.float32, tag="at", bufs=3)
            state[h][4] = at
            for si in range(n_sblk):
                if si % 2 == 0:
                    ep = epool.tile([P, 2, s], mybir.dt.float8e4, tag="E",
                                    bufs=17)
                    epairs.append(ep)
                for j in range(2):
                    psS = ps.tile([P, s // 2], mybir.dt.float32,
                                  tag="S", bufs=3)
                    for nj in range(2):
                        lo, hi = (0, d) if nj == 0 else (d, 2 * d)
                        c0 = j * (s // 2) + nj * 512
                        nc.tensor.matmul(
                            psS[:, nj * 512:(nj + 1) * 512],
                            ptx[lo:hi, si * P:(si + 1) * P],
                            ptx[lo:hi, c0:c0 + 512],
                            start=True, stop=True)
                    e_half = epairs[-1][:, si % 2,
                                        j * (s // 2):(j + 1) * (s // 2)]
                    zc = zsl[:, 2 * si + j:2 * si + j + 1]
                    # Bresenham spread: never give the slower engine two
                    # consecutive slots (slot stream is consumed in order)
                    use_dve = (ctr[0] * dve_num) % dve_mod < dve_num
                    ctr[0] += 1
                    if use_dve:
                        pt_t = sb.tile([P, s // 2], mybir.dt.float16,
                                       tag="pp", bufs=3)
                        nc.vector._custom_dve(
                            expp_op, out=pt_t, in0=psS,
                            s0=C3p, s1=C2p, imm2=C1p)
                        nc.vector._custom_dve(
                            exps_op, out=e_half, in0=pt_t,
                            s0=1.0, s1=GAMMA, accum_out=zc)
                    else:
                        nc.scalar.activation(
                            e_half, psS,
                            AF.Exp, scale=1.0 / 8.0, bias=shift_sb,
                            accum_out=zc)
            state[h][2] = epairs
            state[h][3] = zsl

        pending = None
        n_groups = (heads + group - 1) // group
        for g in range(n_groups):
            hs = list(range(g * group, min((g + 1) * group, heads)))
            for h in hs:
                emit_sin(h)
            for h in hs:
                emit_qk_exp(h)
                if pending is not None:
                    for idx in range(2 * n_sblk):
                        emit_pv_chunk(pending, idx)
                pending = h
        for idx in range(2 * n_sblk):
            emit_pv_chunk(pending, idx)

    nc.compile()
    return nc


_NC_CACHE = {}


def _get_program(key, **kw):
    if key not in _NC_CACHE:
        _NC_CACHE[key] = build_core_program(**kw)
    return _NC_CACHE[key]


def kernel(x: np.ndarray, mask: np.ndarray, theta: np.ndarray) -> np.ndarray:
    """Full-input entry point: shard across 8 NeuronCores, run, gather."""
    from concourse import bass_utils

    assert x.shape == (B, S, E) and theta.shape == (D,)
    # mask is all-False by construction (fill: zeros); attention is unmasked.

    nc = _get_program("full")

    # [B, S, H, D] -> [B*H, S, D] contiguous per-head slabs, theta folded in
    thp = (theta + np.float32(math.pi / 2.0)).astype(np.float32)
    xh = (
        x.reshape(B, S, H, D).transpose(0, 2, 1, 3) + thp
    ).reshape(B * H, S, D).astype(np.float32)
    # transposed layout with duplicated partitions: [B*H, 128, S]
    xt = np.ascontiguousarray(xh.transpose(0, 2, 1))      # [BH, D, S]
    xt = np.concatenate([xt, xt], axis=1)                  # [BH, 128, S]

    in_maps = [
        {
            "xs": np.ascontiguousarray(
                xh[c * HEADS_PER_CORE:(c + 1) * HEADS_PER_CORE]),
            "xst": np.ascontiguousarray(
                xt[c * HEADS_PER_CORE:(c + 1) * HEADS_PER_CORE]),
        }
        for c in range(N_CORES)
    ]

    global _last_in_maps
    _last_in_maps = in_maps
    res = bass_utils.run_bass_kernel_spmd(nc, in_maps, core_ids=list(range(N_CORES)))
    outs = np.concatenate(
        [res.results[c]["out"] for c in range(N_CORES)], axis=0)  # [BH, D, S]
    zh = np.concatenate(
        [res.results[c]["zs"] for c in range(N_CORES)], axis=0)   # [BH, P, 2*n]
    z = zh[:, :, 0::2] + zh[:, :, 1::2]                  # [BH, P, n_sblk]
    # z[h, p, si] is Z for s = si*128 + p
    zfull = z.transpose(0, 2, 1).reshape(B * H, S)       # [BH, S]
    attn = outs / zfull[:, None, :]                      # [BH, D, S]
    return np.ascontiguousarray(
        attn.reshape(B, H, D, S).transpose(0, 3, 1, 2)
    ).reshape(B, S, E)


# revision 28
# speedup vs baseline: 1.0201x; 1.0201x over previous
"""Trainium2 Bass kernel for quantum-projection multi-head self-attention.

Reference computation (per batch b, head h, with D = 64, H = 16):
    proj = cos(x_heads + theta)                         # [S, D]
    G    = proj @ proj.T / sqrt(D)                      # [S, S]  (symmetric!)
    attn = softmax(G, axis=-1) @ proj                   # [S, D]

Sharding: the 64 (b, h) pairs are data-parallel; 8 pairs per NeuronCore.

Device-side plan per head (S = 2048, D = 64), ~331 us vs the 393 us
baseline on the same harness:
  1. Host folds theta' = theta + pi/2 into x and ships it twice:
     natural [S, D] (s mod 128 partitions) AND transposed with
     duplicated partitions xT_dup [128, S] (partition p = row d mod
     64). The T layout eliminates all PE transposes, their PSUM->SBUF
     copies and the partition-duplication DMA of the old design.
  2. Custom DVE ops (both proven <= 8 ALU stages, short bypass taps;
     longer-lived intermediates and a 2-input PREP variant fault the
     HW DVE even though CoreSim accepts them):
       PREP: y = x' - 2pi*round(x'/2pi) -> fp16  (range reduction)
       SINP: sin(y) ~ y(1 + s(c3 + s(c5 + s c7))), s = y^2 (fp16 in,
             2x DVE port mode) -> ptx bf16 [128,S] / pvx fp8e4 [128,nd]
       EXPP: p ~ e^{g/256}-1 deg-3 (reads PSUM fp32) -> fp16
       EXPS: E = ((1+p)^2 gamma)^16 = e^{g/8-4} -> fp8e4, row-sum Z
             via accum_out for free
  3. QK: G[si] = ptx.T @ ptx per 128-row query block, two K=64 row
     groups concurrently on the PE, into a 3-deep rotation of
     [128, 1024] PSUM half-slab pool tiles (6 banks; PV owns 2).
     Separate pool tiles per slot are essential: a single manually
     sliced PSUM tile is dependency-tracked as one object and
     degrades the pipeline to lock-step (+250 us).
  4. exp per half-slab, interleaved 1:2 between DVE (2-pass custom,
     holds the slot only for pass1) and ACT (HW Exp + accumulator
     read for Z). The 1/3 spread keeps consecutive slots on
     different engines; consecutive same-engine slots serialize the
     slot stream.
  5. PV: fp8e4 DoubleRow matmuls contract two 128-key tiles per
     instruction (rhs = E pairs [128, 2, S] written directly by the
     exps): attnT[64, 512] += pvx[:, 2j:2j+2, :].T @ Epair. 2x PE
     throughput, half the instructions. M = 64 only: dual-fp8
     ldweights restricts lhsT free size to <= 128, which is why Z
     rides accum_out instead of a ones column.
  6. DVE copies PSUM->SBUF; outputs (attnT + Z halves) leave via the
     idle GPSIMD queue so their sem waits never block the SP queue
     that feeds input loads. Host divides by Z and transposes.

Emission is software-pipelined one head deep (QK+exp of head h before
PV of head h-1, PV as a contiguous block - interleaving PV into the
QK stream head-of-line-blocks the in-order PE queue). Sins are
batched per GROUP heads with their input DMAs fused per head.
"""

import math
from contextlib import ExitStack

import numpy as np

import concourse.bass as bass
import concourse.mybir as mybir
import concourse.tile as tile
from concourse import bacc

AF = mybir.ActivationFunctionType
ALU = mybir.AluOpType

B, S, E = 4, 2048, 1024
H = 16
D = E // H          # 64
N_CORES = 8
HEADS_PER_CORE = (B * H) // N_CORES  # 8

P = 128             # partitions
MAGIC = 1.5 * 2.0**23   # fp32 round-to-nearest trick constant
TWO_PI = 2.0 * math.pi
SHIFT = 4.0         # exp(g/8 - SHIFT); host multiplies back via Z division

_OPS = None


def _register_ops():
    """Register custom DVE ops at runtime (idempotent).

    PREP_ANTK : u = w - round(w), w = Src0*C0 + Src1  (5 ALU stages)
    EXPP_ANTK : p = ((C0 g + C1) g + C2) g  ~ e^{g/256}-1  (5 stages)
    EXPS_ANTK : out = ((1+p)^2 * C1)^16, accum_out = row sum  (7 stages)
    """
    global _OPS
    if _OPS is not None:
        return _OPS
    import concourse.dve_ops as dops
    from concourse.dve_spec import Spec, Src0, Src1, C0, C1, C2, sq, lower
    from concourse.dve_spec import _has_src1 as has_src1
    from concourse.dve_uop import DveOpSpec, AluOp as UAluOp

    def make_op(name, spec):
        if name in dops._SUB_OPCODE_FOR_NAME:
            return next(o for o in dops.OPS if o.name == name)
        row = 1 + len(dops.OPS)
        assert row < 0x20
        dops._SUB_OPCODE_FOR_NAME[name] = row
        shas = {}
        for ver in ("v3", "v4"):
            uops = lower(spec, ver=ver)
            shas[ver] = DveOpSpec(
                name=name, opcode=row, uops=uops, rd1_en=has_src1(spec)
            ).sha(ver)
        op = dops.DveOp(name, spec, subdim=False, uops_sha=shas)
        dops.OPS.append(op)
        dops.CUSTOM_DVE_SPECS[name] = spec
        return op

    def ref_prep(in0, in1, s0, s1, imm2):
        w = in0.astype(np.float32) * s0
        r = (w + s1) - s1
        return in0.astype(np.float32) - imm2 * r

    # theta is folded into x host-side (x' = x + theta + pi/2), so this
    # is pure range reduction: y = x' - 2pi*round(x'/2pi) in [-pi, pi].
    prep = make_op(
        "PREP_ANTK3",
        Spec(body=Src0 - (((Src0 * C0) + C1) - C1) * C2, reference=ref_prep),
    )

    def ref_sinp(in0, in1, s0, s1, imm2):
        y = in0.astype(np.float32)
        s = y * y
        return y * ((((s0 * s) + s1) * s + imm2) * s + 1.0)

    # sin(y) ~ y*(1 + s*(c3 + s*(c5 + s*c7))), s = y^2, y in [-pi, pi]
    _s = sq(Src0)
    sinp = make_op(
        "SINP_ANTK",
        Spec(body=(_s * ((C0 * _s + C1) * _s + C2)) * Src0 + Src0,
             reference=ref_sinp),
    )

    def ref_expp(in0, in1, s0, s1, imm2):
        g = in0.astype(np.float32)
        return ((s0 * g + s1) * g + imm2) * g

    expp = make_op(
        "EXPP_ANTK",
        Spec(body=((C0 * Src0 + C1) * Src0 + C2) * Src0, reference=ref_expp),
    )

    def ref_exps(in0, in1, s0, s1, imm2):
        q = in0.astype(np.float32) + s0
        q = q * q * s1
        for _ in range(4):
            q = q * q
        return q, q.sum(axis=-1, keepdims=True)

    exps = make_op(
        "EXPS_ANTK",
        Spec(body=sq(sq(sq(sq(sq(Src0 + C0) * C1)))),
             accum=UAluOp.ADD, reference=ref_exps),
    )
    _OPS = (prep, sinp, expp, exps)
    return _OPS


def _sin_poly_coeffs():
    """c7, c5, c3 for sin(y) ~ y*(1 + s*(c3 + s*(c5 + s*c7))), s=y^2."""
    y = np.linspace(-np.pi, np.pi, 20001)
    s = y * y
    mask = np.abs(y) > 1e-6
    A = np.stack([s[mask] ** 3, s[mask] ** 2, s[mask]], 1)
    b = np.sin(y[mask]) / y[mask] - 1.0
    cf, *_ = np.linalg.lstsq(A, b, rcond=None)
    return [float(v) for v in cf]   # c7, c5, c3


def _exp_poly_coeffs():
    """Deg-3 lstsq fit of e^{g/256}-1 (relative-error weighted), |g|<=66."""
    g = np.linspace(-66.0, 66.0, 4001)
    t = np.expm1(g / 256.0)
    A = np.stack([g**3, g**2, g], 1)
    w = 1.0 / np.exp(g / 256.0)
    cf, *_ = np.linalg.lstsq(A * w[:, None], t * w, rcond=None)
    return [float(v) for v in cf]   # c3, c2, c1


def build_core_program(s=S, d=D, heads=HEADS_PER_CORE, group=4,
                       dve_num=1, dve_mod=3, copy_on_act=True):
    """Build the single-core Bass program (same NEFF runs SPMD on all cores).

    Input DRAM tensors (theta + pi/2 is folded into x host-side):
      xs  : [heads, s, d] fp32    (natural per-head x + theta')
      xst : [heads, P, s] fp32    (same, transposed, d duplicated into 64..127)
    Outputs:
      out : [heads, d, s] fp32    (unnormalized attn^T)
      zs  : [heads, P, 2*(s//P)] fp32  (Z half-sums; host adds pairs)
    """
    n_sblk = s // P                   # 16 query blocks of 128 rows
    nd = n_sblk * d                   # 1024
    assert s % P == 0 and d == 64

    nc = bacc.Bacc("TRN2", target_bir_lowering=False, debug=False)

    xs = nc.dram_tensor("xs", [heads, s, d], mybir.dt.float32, kind="ExternalInput")
    xst = nc.dram_tensor("xst", [heads, P, s], mybir.dt.float32, kind="ExternalInput")
    out = nc.dram_tensor("out", [heads, d, s], mybir.dt.float32, kind="ExternalOutput")
    zs = nc.dram_tensor("zs", [heads, P, 2 * n_sblk], mybir.dt.float32,
                        kind="ExternalOutput")

    prep_op, sinp_op, expp_op, exps_op = _register_ops()
    C3p, C2p, C1p = _exp_poly_coeffs()
    S7, S5, S3 = _sin_poly_coeffs()
    GAMMA = math.exp(-SHIFT / 16.0)   # ((1+p)^2*GAMMA)^16 = e^{g/8-SHIFT}

    with tile.TileContext(nc) as tc, ExitStack() as ctx:
        const = ctx.enter_context(tc.tile_pool(name="const", bufs=1))
        sb = ctx.enter_context(tc.tile_pool(name="sb", bufs=2))
        epool = ctx.enter_context(tc.tile_pool(name="epool", bufs=2))
        ps = ctx.enter_context(tc.tile_pool(name="ps", bufs=1, space="PSUM"))

        shift_sb = const.tile([P, 1], mybir.dt.float32, tag="shift")
        nc.vector.memset(shift_sb, -SHIFT)

        # PSUM: 3 rotating QK half-slabs (6 banks) + 2 PV slots (2 banks);
        # separate pool tiles per slot so WAR tracking stays per-slot
        psPV = ps.tile([d, 2, 512], mybir.dt.float32, tag="pv", bufs=1)

        state = {}

        def emit_sin(h):
            xt_t = sb.tile([P, s], mybir.dt.float32, tag="xt", bufs=3)
            xv = xt_t.rearrange("p (q c) -> p q c", q=4)
            xr = xst[h].rearrange("p (q c) -> p q c", q=4)
            for q in range(4):
                nc.sync.dma_start(xv[:, q, :], xr[:, q, :])
            xn_t = sb.tile([P, nd], mybir.dt.float32, tag="xn", bufs=3)
            nv = xn_t.rearrange("p (n d) -> p n d", d=d)
            nr = xs[h].rearrange("(n p) d -> p n d", p=P)
            for q in range(4):
                nc.sync.dma_start(nv[:, q * 4:(q + 1) * 4, :],
                                  nr[:, q * 4:(q + 1) * 4, :])
            ut_t = sb.tile([P, s], mybir.dt.float16, tag="ut", bufs=2)
            nc.vector._custom_dve(
                prep_op, out=ut_t, in0=xt_t,
                s0=1.0 / TWO_PI, s1=MAGIC, imm2=TWO_PI)
            ptx = sb.tile([P, s], mybir.dt.bfloat16, tag="ptx", bufs=2)
            nc.vector._custom_dve(
                sinp_op, out=ptx, in0=ut_t, s0=S7, s1=S5, imm2=S3)
            un = sb.tile([P, nd], mybir.dt.float16, tag="un", bufs=2)
            nc.vector._custom_dve(
                prep_op, out=un, in0=xn_t,
                s0=1.0 / TWO_PI, s1=MAGIC, imm2=TWO_PI)
            pvx = sb.tile([P, nd], mybir.dt.float8e4, tag="pvx", bufs=2)
            nc.vector._custom_dve(
                sinp_op, out=pvx, in0=un, s0=S7, s1=S5, imm2=S3)
            state[h] = [ptx, pvx, None, None, None]

        ctr = [0]

        def emit_pv_chunk(h, idx):
            ptx, pvx, epairs, zsl, at = state[h]
            pvv = pvx.rearrange("p (t d) -> p t d", d=d)
            sb_i, tjp = idx // (n_sblk // 2), idx % (n_sblk // 2)
            slot = sb_i % 2
            nc.tensor.matmul(
                psPV[:, slot, :],
                pvv[:, 2 * tjp:2 * tjp + 2, :],
                epairs[tjp][:, :, sb_i * 512:(sb_i + 1) * 512],
                start=(tjp == 0), stop=(tjp == n_sblk // 2 - 1),
                perf_mode=mybir.MatmulPerfMode.DoubleRow)
            if tjp == n_sblk // 2 - 1:
                cp = nc.scalar.copy if copy_on_act else nc.vector.tensor_copy
                cp(at[:, sb_i * 512:(sb_i + 1) * 512], psPV[:, slot, :])
                if sb_i % 2 == 1:
                    c = sb_i // 2
                    nc.gpsimd.dma_start(
                        out[h, :, c * (s // 2):(c + 1) * (s // 2)],
                        at[:, c * (s // 2):(c + 1) * (s // 2)])
                if sb_i == 3:
                    nc.gpsimd.dma_start(zs[h], zsl)
                    del state[h]

        def emit_qk_exp(h):
            ptx, pvx, _, _, _ = state[h]
            epairs = []
            zsl = sb.tile([P, 2 * n_sblk], mybir.dt.float32, tag="z", bufs=2)
            at = sb.tile([d, s], mybir.dt.float32, tag="at", bufs=3)
            state[h][4] = at
            for si in range(n_sblk):
                if si % 2 == 0:
                    ep = epool.tile([P, 2, s], mybir.dt.float8e4, tag="E",
                                    bufs=17)
                    epairs.append(ep)
                for j in range(2):
                    psS = ps.tile([P, s // 2], mybir.dt.float32,
                                  tag="S", bufs=3)
                    for nj in range(2):
                        lo, hi = (0, d) if nj == 0 else (d, 2 * d)
                        c0 = j * (s // 2) + nj * 512
                        nc.tensor.matmul(
                            psS[:, nj * 512:(nj + 1) * 512],
                            ptx[lo:hi, si * P:(si + 1) * P],
                            ptx[lo:hi, c0:c0 + 512],
                            start=True, stop=True)
                    e_half = epairs[-1][:, si % 2,
                                        j * (s // 2):(j + 1) * (s // 2)]
                    zc = zsl[:, 2 * si + j:2 * si + j + 1]
                    # Bresenham spread: never give the slower engine two
                    # consecutive slots (slot stream is consumed in order)
                    use_dve = (ctr[0] * dve_num) % dve_mod < dve_num
                    ctr[0] += 1
                    if use_dve:
                        pt_t = sb.tile([P, s // 2], mybir.dt.float16,
                                       tag="pp", bufs=3)
                        nc.vector._custom_dve(
                            expp_op, out=pt_t, in0=psS,
                            s0=C3p, s1=C2p, imm2=C1p)
                        nc.vector._custom_dve(
                            exps_op, out=e_half, in0=pt_t,
                            s0=1.0, s1=GAMMA, accum_out=zc)
                    else:
                        nc.scalar.activation(
                            e_half, psS,
                            AF.Exp, scale=1.0 / 8.0, bias=shift_sb,
                            accum_out=zc)
            state[h][2] = epairs
            state[h][3] = zsl

        pending = None
        n_groups = (heads + group - 1) // group
        for g in range(n_groups):
            hs = list(range(g * group, min((g + 1) * group, heads)))
            for h in hs:
                emit_sin(h)
            for h in hs:
                emit_qk_exp(h)
                if pending is not None:
                    for idx in range(2 * n_sblk):
                        emit_pv_chunk(pending, idx)
                pending = h
        for idx in range(2 * n_sblk):
            emit_pv_chunk(pending, idx)

    nc.compile()
    return nc


_NC_CACHE = {}


def _get_program(key, **kw):
    if key not in _NC_CACHE:
        _NC_CACHE[key] = build_core_program(**kw)
    return _NC_CACHE[key]


def kernel(x: np.ndarray, mask: np.ndarray, theta: np.ndarray) -> np.ndarray:
    """Full-input entry point: shard across 8 NeuronCores, run, gather."""
    from concourse import bass_utils

    assert x.shape == (B, S, E) and theta.shape == (D,)
    # mask is all-False by construction (fill: zeros); attention is unmasked.

    nc = _get_program("full")

    # [B, S, H, D] -> [B*H, S, D] contiguous per-head slabs, theta folded in
    thp = (theta + np.float32(math.pi / 2.0)).astype(np.float32)
    xh = (
        x.reshape(B, S, H, D).transpose(0, 2, 1, 3) + thp
    ).reshape(B * H, S, D).astype(np.float32)
    # transposed layout with duplicated partitions: [B*H, 128, S]
    xt = np.ascontiguousarray(xh.transpose(0, 2, 1))      # [BH, D, S]
    xt = np.concatenate([xt, xt], axis=1)                  # [BH, 128, S]

    in_maps = [
        {
            "xs": np.ascontiguousarray(
                xh[c * HEADS_PER_CORE:(c + 1) * HEADS_PER_CORE]),
            "xst": np.ascontiguousarray(
                xt[c * HEADS_PER_CORE:(c + 1) * HEADS_PER_CORE]),
        }
        for c in range(N_CORES)
    ]

    global _last_in_maps
    _last_in_maps = in_maps
    res = bass_utils.run_bass_kernel_spmd(nc, in_maps, core_ids=list(range(N_CORES)))
    outs = np.concatenate(
        [res.results[c]["out"] for c in range(N_CORES)], axis=0)  # [BH, D, S]
    zh = np.concatenate(
        [res.results[c]["zs"] for c in range(N_CORES)], axis=0)   # [BH, P, 2*n]
    z = zh[:, :, 0::2] + zh[:, :, 1::2]                  # [BH, P, n_sblk]
    # z[h, p, si] is Z for s = si*128 + p
    zfull = z.transpose(0, 2, 1).reshape(B * H, S)       # [BH, S]
    attn = outs / zfull[:, None, :]                      # [BH, D, S]
    return np.ascontiguousarray(
        attn.reshape(B, H, D, S).transpose(0, 3, 1, 2)
    ).reshape(B, S, E)


# revision 29
# speedup vs baseline: 1.0247x; 1.0046x over previous
"""Trainium2 Bass kernel for quantum-projection multi-head self-attention.

Reference computation (per batch b, head h, with D = 64, H = 16):
    proj = cos(x_heads + theta)                         # [S, D]
    G    = proj @ proj.T / sqrt(D)                      # [S, S]  (symmetric!)
    attn = softmax(G, axis=-1) @ proj                   # [S, D]

Sharding: the 64 (b, h) pairs are data-parallel; 8 pairs per NeuronCore.

Device-side plan per head (S = 2048, D = 64), ~331 us vs the 393 us
baseline on the same harness:
  1. Host folds theta' = theta + pi/2 into x and ships it twice:
     natural [S, D] (s mod 128 partitions) AND transposed with
     duplicated partitions xT_dup [128, S] (partition p = row d mod
     64). The T layout eliminates all PE transposes, their PSUM->SBUF
     copies and the partition-duplication DMA of the old design.
  2. Custom DVE ops (both proven <= 8 ALU stages, short bypass taps;
     longer-lived intermediates and a 2-input PREP variant fault the
     HW DVE even though CoreSim accepts them):
       PREP: y = x' - 2pi*round(x'/2pi) -> fp16  (range reduction)
       SINP: sin(y) ~ y(1 + s(c3 + s(c5 + s c7))), s = y^2 (fp16 in,
             2x DVE port mode) -> ptx bf16 [128,S] / pvx fp8e4 [128,nd]
       EXPP: p ~ e^{g/256}-1 deg-3 (reads PSUM fp32) -> fp16
       EXPS: E = ((1+p)^2 gamma)^16 = e^{g/8-4} -> fp8e4, row-sum Z
             via accum_out for free
  3. QK: G[si] = ptx.T @ ptx per 128-row query block, two K=64 row
     groups concurrently on the PE, into a 3-deep rotation of
     [128, 1024] PSUM half-slab pool tiles (6 banks; PV owns 2).
     Separate pool tiles per slot are essential: a single manually
     sliced PSUM tile is dependency-tracked as one object and
     degrades the pipeline to lock-step (+250 us).
  4. exp per half-slab, interleaved 1:2 between DVE (2-pass custom,
     holds the slot only for pass1) and ACT (HW Exp + accumulator
     read for Z). The 1/3 spread keeps consecutive slots on
     different engines; consecutive same-engine slots serialize the
     slot stream.
  5. PV: fp8e4 DoubleRow matmuls contract two 128-key tiles per
     instruction (rhs = E pairs [128, 2, S] written directly by the
     exps): attnT[64, 512] += pvx[:, 2j:2j+2, :].T @ Epair. 2x PE
     throughput, half the instructions. M = 64 only: dual-fp8
     ldweights restricts lhsT free size to <= 128, which is why Z
     rides accum_out instead of a ones column.
  6. DVE copies PSUM->SBUF; outputs (attnT + Z halves) leave via the
     idle GPSIMD queue so their sem waits never block the SP queue
     that feeds input loads. Host divides by Z and transposes.

Emission is software-pipelined one head deep (QK+exp of head h before
PV of head h-1, PV as a contiguous block - interleaving PV into the
QK stream head-of-line-blocks the in-order PE queue). Sins are
batched per GROUP heads with their input DMAs fused per head.
"""

import math
from contextlib import ExitStack

import numpy as np

import concourse.bass as bass
import concourse.mybir as mybir
import concourse.tile as tile
from concourse import bacc

AF = mybir.ActivationFunctionType
ALU = mybir.AluOpType

B, S, E = 4, 2048, 1024
H = 16
D = E // H          # 64
N_CORES = 8
HEADS_PER_CORE = (B * H) // N_CORES  # 8

P = 128             # partitions
MAGIC = 1.5 * 2.0**23   # fp32 round-to-nearest trick constant
TWO_PI = 2.0 * math.pi
SHIFT = 4.0         # exp(g/8 - SHIFT); host multiplies back via Z division

_OPS = None


def _register_ops():
    """Register custom DVE ops at runtime (idempotent).

    PREP_ANTK : u = w - round(w), w = Src0*C0 + Src1  (5 ALU stages)
    EXPP_ANTK : p = ((C0 g + C1) g + C2) g  ~ e^{g/256}-1  (5 stages)
    EXPS_ANTK : out = ((1+p)^2 * C1)^16, accum_out = row sum  (7 stages)
    """
    global _OPS
    if _OPS is not None:
        return _OPS
    import concourse.dve_ops as dops
    from concourse.dve_spec import Spec, Src0, Src1, C0, C1, C2, sq, lower
    from concourse.dve_spec import _has_src1 as has_src1
    from concourse.dve_uop import DveOpSpec, AluOp as UAluOp

    def make_op(name, spec):
        if name in dops._SUB_OPCODE_FOR_NAME:
            return next(o for o in dops.OPS if o.name == name)
        row = 1 + len(dops.OPS)
        assert row < 0x20
        dops._SUB_OPCODE_FOR_NAME[name] = row
        shas = {}
        for ver in ("v3", "v4"):
            uops = lower(spec, ver=ver)
            shas[ver] = DveOpSpec(
                name=name, opcode=row, uops=uops, rd1_en=has_src1(spec)
            ).sha(ver)
        op = dops.DveOp(name, spec, subdim=False, uops_sha=shas)
        dops.OPS.append(op)
        dops.CUSTOM_DVE_SPECS[name] = spec
        return op

    def ref_prep(in0, in1, s0, s1, imm2):
        w = in0.astype(np.float32) * s0
        r = (w + s1) - s1
        return in0.astype(np.float32) - imm2 * r

    # theta is folded into x host-side (x' = x + theta + pi/2), so this
    # is pure range reduction: y = x' - 2pi*round(x'/2pi) in [-pi, pi].
    prep = make_op(
        "PREP_ANTK3",
        Spec(body=Src0 - (((Src0 * C0) + C1) - C1) * C2, reference=ref_prep),
    )

    def ref_sinp(in0, in1, s0, s1, imm2):
        y = in0.astype(np.float32)
        s = y * y
        return y * ((((s0 * s) + s1) * s + imm2) * s + 1.0)

    # sin(y) ~ y*(1 + s*(c3 + s*(c5 + s*c7))), s = y^2, y in [-pi, pi]
    _s = sq(Src0)
    sinp = make_op(
        "SINP_ANTK",
        Spec(body=(_s * ((C0 * _s + C1) * _s + C2)) * Src0 + Src0,
             reference=ref_sinp),
    )

    def ref_expp(in0, in1, s0, s1, imm2):
        g = in0.astype(np.float32)
        return ((s0 * g + s1) * g + imm2) * g

    expp = make_op(
        "EXPP_ANTK",
        Spec(body=((C0 * Src0 + C1) * Src0 + C2) * Src0, reference=ref_expp),
    )

    def ref_exps(in0, in1, s0, s1, imm2):
        q = in0.astype(np.float32) + s0
        q = q * q * s1
        for _ in range(4):
            q = q * q
        return q, q.sum(axis=-1, keepdims=True)

    exps = make_op(
        "EXPS_ANTK",
        Spec(body=sq(sq(sq(sq(sq(Src0 + C0) * C1)))),
             accum=UAluOp.ADD, reference=ref_exps),
    )
    _OPS = (prep, sinp, expp, exps)
    return _OPS


def _sin_poly_coeffs():
    """c7, c5, c3 for sin(y) ~ y*(1 + s*(c3 + s*(c5 + s*c7))), s=y^2."""
    y = np.linspace(-np.pi, np.pi, 20001)
    s = y * y
    mask = np.abs(y) > 1e-6
    A = np.stack([s[mask] ** 3, s[mask] ** 2, s[mask]], 1)
    b = np.sin(y[mask]) / y[mask] - 1.0
    cf, *_ = np.linalg.lstsq(A, b, rcond=None)
    return [float(v) for v in cf]   # c7, c5, c3


def _exp_poly_coeffs():
    """Deg-3 lstsq fit of e^{g/256}-1 (relative-error weighted), |g|<=66."""
    g = np.linspace(-66.0, 66.0, 4001)
    t = np.expm1(g / 256.0)
    A = np.stack([g**3, g**2, g], 1)
    w = 1.0 / np.exp(g / 256.0)
    cf, *_ = np.linalg.lstsq(A * w[:, None], t * w, rcond=None)
    return [float(v) for v in cf]   # c3, c2, c1


def build_core_program(s=S, d=D, heads=HEADS_PER_CORE, group=4,
                       dve_num=3, dve_mod=8, copy_on_act=True):
    """Build the single-core Bass program (same NEFF runs SPMD on all cores).

    Input DRAM tensors (theta + pi/2 is folded into x host-side):
      xs  : [heads, s, d] fp32    (natural per-head x + theta')
      xst : [heads, P, s] fp32    (same, transposed, d duplicated into 64..127)
    Outputs:
      out : [heads, d, s] fp32    (unnormalized attn^T)
      zs  : [heads, P, 2*(s//P)] fp32  (Z half-sums; host adds pairs)
    """
    n_sblk = s // P                   # 16 query blocks of 128 rows
    nd = n_sblk * d                   # 1024
    assert s % P == 0 and d == 64

    nc = bacc.Bacc("TRN2", target_bir_lowering=False, debug=False)

    xs = nc.dram_tensor("xs", [heads, s, d], mybir.dt.float32, kind="ExternalInput")
    xst = nc.dram_tensor("xst", [heads, P, s], mybir.dt.float32, kind="ExternalInput")
    out = nc.dram_tensor("out", [heads, d, s], mybir.dt.float32, kind="ExternalOutput")
    zs = nc.dram_tensor("zs", [heads, P, 2 * n_sblk], mybir.dt.float32,
                        kind="ExternalOutput")

    prep_op, sinp_op, expp_op, exps_op = _register_ops()
    C3p, C2p, C1p = _exp_poly_coeffs()
    S7, S5, S3 = _sin_poly_coeffs()
    GAMMA = math.exp(-SHIFT / 16.0)   # ((1+p)^2*GAMMA)^16 = e^{g/8-SHIFT}

    with tile.TileContext(nc) as tc, ExitStack() as ctx:
        const = ctx.enter_context(tc.tile_pool(name="const", bufs=1))
        sb = ctx.enter_context(tc.tile_pool(name="sb", bufs=2))
        epool = ctx.enter_context(tc.tile_pool(name="epool", bufs=2))
        ps = ctx.enter_context(tc.tile_pool(name="ps", bufs=1, space="PSUM"))

        shift_sb = const.tile([P, 1], mybir.dt.float32, tag="shift")
        nc.vector.memset(shift_sb, -SHIFT)

        # PSUM: 3 rotating QK half-slabs (6 banks) + 2 PV slots (2 banks);
        # separate pool tiles per slot so WAR tracking stays per-slot
        psPV = ps.tile([d, 2, 512], mybir.dt.float32, tag="pv", bufs=1)

        state = {}

        def emit_sin(h):
            xt_t = sb.tile([P, s], mybir.dt.float32, tag="xt", bufs=3)
            xv = xt_t.rearrange("p (q c) -> p q c", q=4)
            xr = xst[h].rearrange("p (q c) -> p q c", q=4)
            for q in range(4):
                nc.sync.dma_start(xv[:, q, :], xr[:, q, :])
            xn_t = sb.tile([P, nd], mybir.dt.float32, tag="xn", bufs=3)
            nv = xn_t.rearrange("p (n d) -> p n d", d=d)
            nr = xs[h].rearrange("(n p) d -> p n d", p=P)
            for q in range(4):
                nc.sync.dma_start(nv[:, q * 4:(q + 1) * 4, :],
                                  nr[:, q * 4:(q + 1) * 4, :])
            ut_t = sb.tile([P, s], mybir.dt.float16, tag="ut", bufs=2)
            nc.vector._custom_dve(
                prep_op, out=ut_t, in0=xt_t,
                s0=1.0 / TWO_PI, s1=MAGIC, imm2=TWO_PI)
            ptx = sb.tile([P, s], mybir.dt.bfloat16, tag="ptx", bufs=2)
            nc.vector._custom_dve(
                sinp_op, out=ptx, in0=ut_t, s0=S7, s1=S5, imm2=S3)
            un = sb.tile([P, nd], mybir.dt.float16, tag="un", bufs=2)
            nc.vector._custom_dve(
                prep_op, out=un, in0=xn_t,
                s0=1.0 / TWO_PI, s1=MAGIC, imm2=TWO_PI)
            pvx = sb.tile([P, nd], mybir.dt.float8e4, tag="pvx", bufs=2)
            nc.vector._custom_dve(
                sinp_op, out=pvx, in0=un, s0=S7, s1=S5, imm2=S3)
            state[h] = [ptx, pvx, None, None, None]

        ctr = [0]

        def emit_pv_chunk(h, idx):
            ptx, pvx, epairs, zsl, at = state[h]
            pvv = pvx.rearrange("p (t d) -> p t d", d=d)
            sb_i, tjp = idx // (n_sblk // 2), idx % (n_sblk // 2)
            slot = sb_i % 2
            nc.tensor.matmul(
                psPV[:, slot, :],
                pvv[:, 2 * tjp:2 * tjp + 2, :],
                epairs[tjp][:, :, sb_i * 512:(sb_i + 1) * 512],
                start=(tjp == 0), stop=(tjp == n_sblk // 2 - 1),
                perf_mode=mybir.MatmulPerfMode.DoubleRow)
            if tjp == n_sblk // 2 - 1:
                cp = nc.scalar.copy if copy_on_act else nc.vector.tensor_copy
                cp(at[:, sb_i * 512:(sb_i + 1) * 512], psPV[:, slot, :])
                if sb_i % 2 == 1:
                    c = sb_i // 2
                    nc.gpsimd.dma_start(
                        out[h, :, c * (s // 2):(c + 1) * (s // 2)],
                        at[:, c * (s // 2):(c + 1) * (s // 2)])
                if sb_i == 3:
                    nc.gpsimd.dma_start(zs[h], zsl)
                    del state[h]

        def emit_qk_exp(h):
            ptx, pvx, _, _, _ = state[h]
            epairs = []
            zsl = sb.tile([P, 2 * n_sblk], mybir.dt.float32, tag="z", bufs=2)
            at = sb.tile([d, s], mybir.dt.float32, tag="at", bufs=3)
            state[h][4] = at
            for si in range(n_sblk):
                if si % 2 == 0:
                    ep = epool.tile([P, 2, s], mybir.dt.float8e4, tag="E",
                                    bufs=17)
                    epairs.append(ep)
                for j in range(2):
                    psS = ps.tile([P, s // 2], mybir.dt.float32,
                                  tag="S", bufs=3)
                    for nj in range(2):
                        lo, hi = (0, d) if nj == 0 else (d, 2 * d)
                        c0 = j * (s // 2) + nj * 512
                        nc.tensor.matmul(
                            psS[:, nj * 512:(nj + 1) * 512],
                            ptx[lo:hi, si * P:(si + 1) * P],
                            ptx[lo:hi, c0:c0 + 512],
                            start=True, stop=True)
                    e_half = epairs[-1][:, si % 2,
                                        j * (s // 2):(j + 1) * (s // 2)]
                    zc = zsl[:, 2 * si + j:2 * si + j + 1]
                    # Bresenham spread: never give the slower engine two
                    # consecutive slots (slot stream is consumed in order)
                    use_dve = (ctr[0] * dve_num) % dve_mod < dve_num
                    ctr[0] += 1
                    if use_dve:
                        pt_t = sb.tile([P, s // 2], mybir.dt.float16,
                                       tag="pp", bufs=3)
                        nc.vector._custom_dve(
                            expp_op, out=pt_t, in0=psS,
                            s0=C3p, s1=C2p, imm2=C1p)
                        nc.vector._custom_dve(
                            exps_op, out=e_half, in0=pt_t,
                            s0=1.0, s1=GAMMA, accum_out=zc)
                    else:
                        nc.scalar.activation(
                            e_half, psS,
                            AF.Exp, scale=1.0 / 8.0, bias=shift_sb,
                            accum_out=zc)
            state[h][2] = epairs
            state[h][3] = zsl

        pending = None
        n_groups = (heads + group - 1) // group
        for g in range(n_groups):
            hs = list(range(g * group, min((g + 1) * group, heads)))
            for h in hs:
                emit_sin(h)
            for h in hs:
                emit_qk_exp(h)
                if pending is not None:
                    for idx in range(2 * n_sblk):
                        emit_pv_chunk(pending, idx)
                pending = h
        for idx in range(2 * n_sblk):
            emit_pv_chunk(pending, idx)

    nc.compile()
    return nc


_NC_CACHE = {}


def _get_program(key, **kw):
    if key not in _NC_CACHE:
        _NC_CACHE[key] = build_core_program(**kw)
    return _NC_CACHE[key]


def kernel(x: np.ndarray, mask: np.ndarray, theta: np.ndarray) -> np.ndarray:
    """Full-input entry point: shard across 8 NeuronCores, run, gather."""
    from concourse import bass_utils

    assert x.shape == (B, S, E) and theta.shape == (D,)
    # mask is all-False by construction (fill: zeros); attention is unmasked.

    nc = _get_program("full")

    # [B, S, H, D] -> [B*H, S, D] contiguous per-head slabs, theta folded in
    thp = (theta + np.float32(math.pi / 2.0)).astype(np.float32)
    xh = (
        x.reshape(B, S, H, D).transpose(0, 2, 1, 3) + thp
    ).reshape(B * H, S, D).astype(np.float32)
    # transposed layout with duplicated partitions: [B*H, 128, S]
    xt = np.ascontiguousarray(xh.transpose(0, 2, 1))      # [BH, D, S]
    xt = np.concatenate([xt, xt], axis=1)                  # [BH, 128, S]

    in_maps = [
        {
            "xs": np.ascontiguousarray(
                xh[c * HEADS_PER_CORE:(c + 1) * HEADS_PER_CORE]),
            "xst": np.ascontiguousarray(
                xt[c * HEADS_PER_CORE:(c + 1) * HEADS_PER_CORE]),
        }
        for c in range(N_CORES)
    ]

    global _last_in_maps
    _last_in_maps = in_maps
    res = bass_utils.run_bass_kernel_spmd(nc, in_maps, core_ids=list(range(N_CORES)))
    outs = np.concatenate(
        [res.results[c]["out"] for c in range(N_CORES)], axis=0)  # [BH, D, S]
    zh = np.concatenate(
        [res.results[c]["zs"] for c in range(N_CORES)], axis=0)   # [BH, P, 2*n]
    z = zh[:, :, 0::2] + zh[:, :, 1::2]                  # [BH, P, n_sblk]
    # z[h, p, si] is Z for s = si*128 + p
    zfull = z.transpose(0, 2, 1).reshape(B * H, S)       # [BH, S]
    attn = outs / zfull[:, None, :]                      # [BH, D, S]
    return np.ascontiguousarray(
        attn.reshape(B, H, D, S).transpose(0, 3, 1, 2)
    ).reshape(B, S, E)
